# revision 13
# baseline (speedup 1.0000x reference)
"""Causal self-attention kernel for 8 TRN2 NeuronCores.

Sharding: data-parallel over batch (B=8 -> 1 batch element per core).
Each core computes full 16-head causal attention for its batch element.
All matmuls run in bf16 with fp32 PSUM accumulation (~5.7e-3 rel err).

Per-core dataflow (L=1024, E=1024, H=16, D=64):
  XT  = x^T            host-pre-transposed bf16, loaded in ct-chunks
                       (first on the sync ring: tiny strided loads after
                       it would stall the FIFO ring for ~10us)
  V   = (x Wv + bv)|1  ct-outer accumulation into 8 concurrent PSUM
                       banks so the PE starts as soon as the first
                       XT/Wv chunks land (instead of after the full 4MB)
  QT  = Wq^T x^T + bq  [e, l] layout (stationary Wq blocks, moving XT)
  KT  = Wk^T x^T + bk  [e, l] layout
  attention            per head PAIR (2et, 2et+1): rounds of
                       [AV pair(kt-1), scores pair(kt)] so the ScalarE
                       exp of kt-1 hides behind the scores of kt; the
                       QK projection of e-tile et+1 is emitted between
                       S(7) and AV(7) so the final exps hide behind it;
                       ones-column in V emits softmax denominators free
  Y   = Yu[0:64]/s     s broadcast via DRAM round-trip + SWDGE bcast +
                       approx reciprocal (last pair: PE ones-matmul
                       broadcast instead, to kill the tail latency)
  out = Y^T.T Wo + bo  contraction over e-tiles; l-tiles 0..3 are woven
                       into the last head pair's rounds (using the PSUM
                       banks freed by the qc0 accumulators); stores in
                       bf16 on alternating HWDGE rings (host casts back)

PSUM budget (8 banks): big pool 2x[128,1024] slots (scores st + proj ps
+ last-pair ones-bcast) = 4, yu accumulators 4x[65,512] = 4 (slots
shared with the woven out-proj ps after their groups close).
DMA rings: sync = XT, wo, mask/biases, half the stores; scalar = wv,
wq/wk blocks, other half of stores; SWDGE = bv/bo/denominator bcasts.
"""

import os
import sys

sys.path.insert(0, "/opt/trn_rl_repo")

import numpy as np

import concourse.bass as bass
import concourse.mybir as mybir
import concourse.tile as tile
from concourse import bacc
from concourse.bass_utils import run_bass_kernel_spmd
f32 = mybir.dt.float32
bf16 = mybir.dt.bfloat16
AF = mybir.ActivationFunctionType
OP = mybir.AluOpType

L = 1024
E = 1024
H = 16
D = 64
P = 128
NT = L // P  # 8 tiles along any 1024 dim
SCALE = 1.0 / np.sqrt(D)


def _build():
    nc = bacc.Bacc("TRN2", target_bir_lowering=False, debug=False, num_devices=8)
    wq = nc.dram_tensor("wq", [NT, P, NT, P], bf16, kind="ExternalInput").ap()
    wk = nc.dram_tensor("wk", [NT, P, NT, P], bf16, kind="ExternalInput").ap()
    wv = nc.dram_tensor("wv", [P, NT, E], bf16, kind="ExternalInput").ap()
    wo = nc.dram_tensor("wo", [P, NT, E], bf16, kind="ExternalInput").ap()
    bqr = nc.dram_tensor("bqr", [P, NT], f32, kind="ExternalInput").ap()
    bkr = nc.dram_tensor("bkr", [P, NT], f32, kind="ExternalInput").ap()
    bv = nc.dram_tensor("bv", [E], f32, kind="ExternalInput").ap()
    bo = nc.dram_tensor("bo", [E], f32, kind="ExternalInput").ap()
    xt_d = nc.dram_tensor("xt", [P, NT, L], bf16, kind="ExternalInput").ap()
    mask_d = nc.dram_tensor("mask01", [P, P], bf16, kind="ExternalInput").ap()
    out = nc.dram_tensor("out", [L, E], bf16, kind="ExternalOutput").ap()
    s_dram = nc.dram_tensor("s_scratch", [H, L], f32, kind="Internal").ap()

    with tile.TileContext(nc) as tc:
        _body(nc, tc, wq, wk, wv, wo, bqr, bkr, bv, bo, out, s_dram,
              xt_d, mask_d)
    return nc


def _body(nc, tc, wq, wk, wv, wo, bqr, bkr, bv, bo, out, s_dram, xt_d, mask_d):
    from contextlib import ExitStack

    ctx = ExitStack()
    with ctx:
        consts = ctx.enter_context(tc.tile_pool(name="consts", bufs=1))
        xt_pool = ctx.enter_context(tc.tile_pool(name="xt_pool", bufs=1))
        qt_pool = ctx.enter_context(tc.tile_pool(name="qt_pool", bufs=1))
        kt_pool = ctx.enter_context(tc.tile_pool(name="kt_pool", bufs=1))
        v_pool = ctx.enter_context(tc.tile_pool(name="v_pool", bufs=1))
        y_pool = ctx.enter_context(tc.tile_pool(name="y_pool", bufs=1))
        sst_pool = ctx.enter_context(tc.tile_pool(name="sst_pool", bufs=4))
        wblk_pool = ctx.enter_context(tc.tile_pool(name="wblk_pool", bufs=4))
        pt_pool = ctx.enter_context(tc.tile_pool(name="pt_pool", bufs=4))
        osb_pool = ctx.enter_context(tc.tile_pool(name="osb_pool", bufs=3))
        r_pool = ctx.enter_context(tc.tile_pool(name="r_pool", bufs=1))
        rh_pool = ctx.enter_context(tc.tile_pool(name="rh_pool", bufs=3))
        wo_pool = ctx.enter_context(tc.tile_pool(name="wo_pool", bufs=1))

        XT = xt_pool.tile([P, NT, L], bf16)  # [p, ct, l] = x^T[ct*128+p, l]
        QT = qt_pool.tile([P, NT, L], bf16)  # [p, et, l] = Q^T[et*128+p, l]
        KT = kt_pool.tile([P, NT, L], bf16)
        V = v_pool.tile([P, NT, H, D + 1], bf16)  # [p(l), lt, h, d | ones]
        Y = y_pool.tile([P, NT, L], bf16)  # [p, et, l] = y^T[et*128+p, l]
        Ybc = Y
        R = r_pool.tile([P, NT, L], f32)
        wo_r = wo_pool.tile([P, NT, E], bf16)

        # sync ring: XT chunks FIRST, then wo, then the small consts
        mask01 = consts.tile([P, P], bf16)
        ones_t = consts.tile([D + 1, P], bf16)
        bq_sb = consts.tile([P, NT], f32)
        bk_sb = consts.tile([P, NT], f32)
        bv_bc = consts.tile([P, E], f32)
        bo_bc = consts.tile([P, E], f32)

        nc.vector.memset(ones_t, 0.0)
        nc.vector.memset(ones_t[D : D + 1, :], 1.0)
        nc.vector.memset(V[:, :, :, D : D + 1], 1.0)
        nc.gpsimd.dma_start(
            out=bv_bc,
            in_=bass.AP(tensor=bv.tensor, offset=bv.offset, ap=[[0, P], [1, E]]),
        )

        # ---- Phase 1+2a: chunked XT/Wv loads + ct-outer V projection ----
        with tc.tile_pool(name="wv_pool", bufs=1) as wvp, \
             tc.tile_pool(name="vps", bufs=1, space="PSUM") as vps:
            wv_sb = wvp.tile([P, NT, E], bf16)
            for ct in range(NT):
                nc.sync.dma_start(out=XT[:, ct, :], in_=xt_d[:, ct, :])
                nc.scalar.dma_start(out=wv_sb[:, ct, :], in_=wv[:, ct, :])
            nc.sync.dma_start(out=wo_r, in_=wo)
            nc.sync.dma_start(out=mask01, in_=mask_d)
            nc.sync.dma_start(out=bq_sb, in_=bqr)
            nc.sync.dma_start(out=bk_sb, in_=bkr)
            for ec in range(2):
                psv = [
                    vps.tile([P, 512], f32, tag=f"v{lt}", name=f"psv{lt}")
                    for lt in range(NT)
                ]
                for ct in range(NT):
                    for lt in range(NT):
                        nc.tensor.matmul(
                            psv[lt],
                            XT[:, ct, lt * P : (lt + 1) * P],
                            wv_sb[:, ct, ec * 512 : (ec + 1) * 512],
                            start=(ct == 0),
                            stop=(ct == NT - 1),
                        )
                for lt in range(NT):
                    nc.vector.tensor_tensor(
                        out=V[:, lt, ec * 8 : (ec + 1) * 8, 0:D],
                        in0=psv[lt].rearrange("p (h d) -> p h d", h=8),
                        in1=bv_bc[:, ec * 512 : (ec + 1) * 512].rearrange(
                            "p (h d) -> p h d", h=8
                        ),
                        op=OP.add,
                    )

        # "big": 2x 2-bank slots shared by scores st / proj ps / ones-bcast
        big = ctx.enter_context(tc.tile_pool(name="big", bufs=2, space="PSUM"))
        yph = ctx.enter_context(tc.tile_pool(name="yph", bufs=1, space="PSUM"))
        yph2 = ctx.enter_context(tc.tile_pool(name="yph2", bufs=1, space="PSUM"))
        yps = (yph, yph2)

        def emit_proj(et):
            for (w_dram, b_sb, dst) in ((wq, bq_sb, QT), (wk, bk_sb, KT)):
                wqk_blk = wblk_pool.tile(
                    [P, NT, P], bf16, tag="wqkblk", name="wqk_blk"
                )
                nc.scalar.dma_start(out=wqk_blk, in_=w_dram[et])
                for lc in range(2):
                    ps = big.tile([P, L], f32, tag="st", name="ps_proj")
                    for ct in range(NT):
                        nc.tensor.matmul(
                            ps[:, 0:512],
                            wqk_blk[:, ct, :],
                            XT[:, ct, lc * 512 : (lc + 1) * 512],
                            start=(ct == 0),
                            stop=(ct == NT - 1),
                        )
                    nc.vector.tensor_scalar(
                        out=dst[:, et, lc * 512 : (lc + 1) * 512],
                        in0=ps[:, 0:512],
                        scalar1=b_sb[:, et : et + 1],
                        scalar2=None,
                        op0=OP.add,
                    )

        def out_proj_lt(lt, woven=False):
            # woven (inside the last pair): only the qc0 accumulator banks
            # are free — qc1 groups are still open until AV(7)
            for oc in range(2):
                ps = yps[oc].tile(
                    [P, 512], f32,
                    tag="yu0" if woven else f"yu{lt % 2}",
                    name="ps_out",
                )
                for et in range(NT):
                    nc.tensor.matmul(
                        ps,
                        Ybc[:, et, lt * P : (lt + 1) * P],
                        wo_r[:, et, oc * 512 : (oc + 1) * 512],
                        start=(et == 0),
                        stop=(et == NT - 1),
                    )
                osb = osb_pool.tile([P, 512], bf16)
                nc.vector.tensor_tensor(
                    out=osb,
                    in0=ps,
                    in1=bo_bc[:, oc * 512 : (oc + 1) * 512],
                    op=OP.add,
                )
                eng = nc.sync if (lt + oc) % 2 == 0 else nc.scalar
                eng.dma_start(
                    out=out[lt * P : (lt + 1) * P, oc * 512 : (oc + 1) * 512],
                    in_=osb,
                )

        # ---- Phase 2b+3: proj(0) prologue, then per-et attention pairs ----
        emit_proj(0)
        for et in range(NT):
            last_pair = et == NT - 1
            if last_pair:
                fillers = {
                    4 + i: (lambda lt=i: out_proj_lt(lt, woven=True))
                    for i in range(4)
                }
            else:
                fillers = {NT - 1: (lambda e=et + 1: emit_proj(e))}
            _attention_pair(
                nc, et, QT, KT, V, Ybc, s_dram, big, yps, pt_pool,
                sst_pool, mask01, last_pair, ones_t, rh_pool, fillers,
            )
            if not last_pair:
                for half in range(2):
                    hh = 2 * et + half
                    bsrc = bass.AP(
                        tensor=s_dram.tensor,
                        offset=s_dram[hh : hh + 1, :].offset,
                        ap=[[0, 64], [1, L]],
                    )
                    nc.gpsimd.dma_start(
                        out=R[half * 64 : (half + 1) * 64, et, :], in_=bsrc
                    )
                nc.vector.reciprocal_approx_fast(out=R[:, et, :], in_=R[:, et, :])
                for half in range(2):
                    rows = slice(half * 64, (half + 1) * 64)
                    nc.vector.tensor_tensor(
                        out=Ybc[rows, et, :],
                        in0=Y[rows, et, :],
                        in1=R[rows, et, :],
                        op=OP.mult,
                    )
                if et == 0:
                    # deferred: not needed before the out-proj, keeps the
                    # SWDGE queue clear of the head's XT/Wv window
                    nc.gpsimd.dma_start(
                        out=bo_bc,
                        in_=bass.AP(
                            tensor=bo.tensor, offset=bo.offset,
                            ap=[[0, P], [1, E]],
                        ),
                    )

        # ---- Phase 5 tail: remaining out-proj l-tiles ----
        for lt in range(4, NT):
            out_proj_lt(lt)


def _attention_pair(nc, et, QT, KT, V, Ybc, s_dram, big, yps, pt_pool,
                    sst_pool, mask01, last_pair, ones_t, rh_pool, fillers):
    """Both heads (2et, 2et+1) of e-tile et, in [AV(kt-1), S(kt)] rounds."""
    heads = (2 * et, 2 * et + 1)
    yu = {}
    for hi in range(2):
        for qc in range(2):
            yu[(hi, qc)] = yps[hi].tile(
                [D + 1, 512], f32, tag=f"yu{qc}", name=f"yu{hi}{qc}"
            )

    pend = None  # pt tiles of kt-1
    for kt in range(NT):
        if pend is not None:
            _emit_av(nc, et, heads, V, kt - 1, pend, yu, Ybc, s_dram,
                     sst_pool, last_pair, ones_t, rh_pool, big)
        qlen = L - kt * P
        pts = []
        for hi in range(2):
            pb = hi * 64
            st = big.tile([P, L], f32, tag="st", name="st")
            for s0 in range(0, qlen, 512):
                n = min(512, qlen - s0)
                nc.tensor.matmul(
                    st[:, s0 : s0 + n],
                    KT[pb : pb + D, et, kt * P : (kt + 1) * P],
                    QT[pb : pb + D, et, kt * P + s0 : kt * P + s0 + n],
                    start=True,
                    stop=True,
                )
            pt = pt_pool.tile([P, L], bf16, tag="pt", name="pt")
            nc.scalar.activation(
                out=pt[:, 0:qlen], in_=st[:, 0:qlen], func=AF.Exp,
                scale=float(SCALE),
            )
            nc.vector.tensor_tensor(
                out=pt[:, 0:P], in0=pt[:, 0:P], in1=mask01, op=OP.mult
            )
            pts.append(pt)
        if kt in fillers:
            fillers[kt]()
        pend = pts
    _emit_av(nc, et, heads, V, NT - 1, pend, yu, Ybc, s_dram,
             sst_pool, last_pair, ones_t, rh_pool, big)


def _emit_av(nc, et, heads, V, kt, pts, yu, Ybc, s_dram, sst_pool, last_pair,
             ones_t, rh_pool, big):
    for hi, h in enumerate(heads):
        for qc in range(2):
            lo = max(qc * 512, kt * P)
            hi_col = (qc + 1) * 512
            if lo >= hi_col:
                continue
            last_kt = min(NT - 1, (qc + 1) * 4 - 1)
            nc.tensor.matmul(
                yu[(hi, qc)][:, lo - qc * 512 : hi_col - qc * 512],
                V[:, kt, h, :],
                pts[hi][:, lo - kt * P : hi_col - kt * P],
                start=(kt == 0),
                stop=(kt == last_kt),
            )
            if kt == last_kt:
                _head_tail(nc, h, hi, qc, yu, Ybc, s_dram, sst_pool,
                           last_pair, ones_t, rh_pool, big)


def _head_tail(nc, h, hi, qc, yu, Ybc, s_dram, sst_pool, last_pair, ones_t,
               rh_pool, big):
    et = h // 2
    pb = hi * 64
    cols = slice(qc * 512, (qc + 1) * 512)
    yuq = yu[(hi, qc)]
    if last_pair:
        sstb = sst_pool.tile([D + 1, 512], bf16, tag="sstb", name="sstb")
        nc.vector.tensor_copy(out=sstb[D : D + 1, :], in_=yuq[D : D + 1, :])
        ps_bc = big.tile([P, L], f32, tag="st", name="ps_bc")
        nc.tensor.matmul(
            ps_bc[:, 0:512], ones_t[D : D + 1, :], sstb[D : D + 1, :],
            start=True, stop=True,
        )
        rh = rh_pool.tile([P, 512], f32, tag="rh", name="rh")
        nc.vector.reciprocal_approx_fast(out=rh, in_=ps_bc[:, 0:512])
        nc.vector.tensor_tensor(
            out=Ybc[pb : pb + D, et, cols],
            in0=yuq[0:D, :],
            in1=rh[0:D, :],
            op=OP.mult,
        )
    else:
        sst = sst_pool.tile([D + 1, 512], f32, tag="sst", name="sst")
        nc.vector.tensor_copy(out=sst[D : D + 1, :], in_=yuq[D : D + 1, :])
        nc.sync.dma_start(out=s_dram[h : h + 1, cols], in_=sst[D : D + 1, :])
        nc.vector.tensor_copy(out=Ybc[pb : pb + D, et, cols], in_=yuq[0:D, :])


_COMPILED = None


def _get_compiled():
    global _COMPILED
    if _COMPILED is None:
        nc = _build()
        nc.compile()
        _COMPILED = nc
    return _COMPILED


def kernel(x, Wq, bq, Wk, bk, Wv, bv, Wo, bo, _trace=False):
    import ml_dtypes

    bfl = ml_dtypes.bfloat16
    nc = _get_compiled()
    x = np.ascontiguousarray(np.asarray(x, dtype=np.float32).astype(bfl))
    B = x.shape[0]
    assert B == 8 and x.shape[1] == L and x.shape[2] == E
    def _qk_layout(w):
        # [et, p, ct, e']: per-et contiguous [128, 8, 128] stationary blocks
        w = np.asarray(w, np.float32).astype(bfl)
        return np.ascontiguousarray(
            w.reshape(NT, P, NT, P).transpose(2, 1, 0, 3)
        )

    def _pct_layout(w):
        # [p, ct, e]: moving-operand blocks with contraction rows on partitions
        w = np.asarray(w, np.float32).astype(bfl)
        return np.ascontiguousarray(w.reshape(NT, P, E).transpose(1, 0, 2))

    def _b_rearranged(b):
        # [p, et]: per-partition bias rows, contiguous for a fast DMA
        return np.ascontiguousarray(
            np.asarray(b, np.float32).reshape(NT, P).T
        )

    common = {
        "wq": _qk_layout(Wq),
        "wk": _qk_layout(Wk),
        "wv": _pct_layout(Wv),
        "wo": _pct_layout(Wo),
        "bqr": _b_rearranged(bq),
        "bkr": _b_rearranged(bk),
        "bv": np.ascontiguousarray(np.asarray(bv, np.float32)),
        "bo": np.ascontiguousarray(np.asarray(bo, np.float32)),
    }
    common["mask01"] = np.tril(np.ones((P, P), np.float32)).T.astype(bfl)
    # xt[b]: [p, ct, l] with xt[b][p, ct, l] = x[b, l, ct*128+p]
    xt = np.ascontiguousarray(
        x.transpose(0, 2, 1).reshape(B, NT, P, L).transpose(0, 2, 1, 3)
    )
    in_maps = [dict(common, xt=xt[i]) for i in range(B)]
    res = run_bass_kernel_spmd(nc, in_maps, core_ids=list(range(8)), trace=_trace)
    outp = np.stack(
        [np.asarray(res.results[i]["out"]).astype(np.float32) for i in range(B)]
    )
    if _trace:
        kernel.last_exec_time_ns = res.exec_time_ns
    return outp


# revision 22
# speedup vs baseline: 1.1602x; 1.1602x over previous
"""Causal self-attention kernel for 8 TRN2 NeuronCores.

Sharding: data-parallel over batch (B=8 -> 1 batch element per core).
Each core computes full 16-head causal attention for its batch element.
All matmuls run in bf16 with fp32 PSUM accumulation (~5.7e-3 rel err).

Per-core dataflow (L=1024, E=1024, H=16, D=64):
  XT  = x^T            host-pre-transposed bf16, loaded in ct-chunks
                       (first on the sync ring: tiny strided loads after
                       it would stall the FIFO ring for ~10us)
  V   = (x Wv + bv)|1  ct-outer accumulation into 8 concurrent PSUM
                       banks so the PE starts as soon as the first
                       XT/Wv chunks land (instead of after the full 4MB)
  QT  = Wq^T x^T + bq  [e, l] layout (stationary Wq blocks, moving XT)
  KT  = Wk^T x^T + bk  [e, l] layout
  attention            per head PAIR (2et, 2et+1): rounds of
                       [AV pair(kt-1), scores pair(kt)] so the ScalarE
                       exp of kt-1 hides behind the scores of kt; the
                       QK projection of e-tile et+1 is emitted between
                       S(7) and AV(7) so the final exps hide behind it;
                       ones-column in V emits softmax denominators free
  Y   = Yu[0:64]/s     s broadcast via DRAM round-trip + SWDGE bcast +
                       approx reciprocal (last pair: PE ones-matmul
                       broadcast instead, to kill the tail latency)
  out = Y^T.T Wo + bo  contraction over e-tiles; l-tiles 0..3 are woven
                       into the last head pair's rounds (using the PSUM
                       banks freed by the qc0 accumulators); stores in
                       bf16 on alternating HWDGE rings (host casts back)

PSUM budget (8 banks): big pool 2x[128,1024] slots (scores st + proj ps
+ last-pair ones-bcast) = 4, yu accumulators 4x[65,512] = 4 (slots
shared with the woven out-proj ps after their groups close).
DMA rings: sync = XT, wo, mask/biases, half the stores; scalar = wv,
wq/wk blocks, other half of stores; SWDGE = bv/bo/denominator bcasts.
"""

import os
import sys

sys.path.insert(0, "/opt/trn_rl_repo")

import numpy as np

import concourse.bass as bass
import concourse.mybir as mybir
import concourse.tile as tile
from concourse import bacc
from concourse.bass_utils import run_bass_kernel_spmd
f32 = mybir.dt.float32
bf16 = mybir.dt.bfloat16
AF = mybir.ActivationFunctionType
OP = mybir.AluOpType

L = 1024
E = 1024
H = 16
D = 64
P = 128
NT = L // P  # 8 tiles along any 1024 dim
SCALE = 1.0 / np.sqrt(D)


def _build():
    nc = bacc.Bacc("TRN2", target_bir_lowering=False, debug=False, num_devices=8)
    wq = nc.dram_tensor("wq", [NT, P, NT, P], bf16, kind="ExternalInput").ap()
    wk = nc.dram_tensor("wk", [NT, P, NT, P], bf16, kind="ExternalInput").ap()
    wv = nc.dram_tensor("wv", [P, NT, E], bf16, kind="ExternalInput").ap()
    wo = nc.dram_tensor("wo", [P, NT, E], bf16, kind="ExternalInput").ap()
    bqr = nc.dram_tensor("bqr", [P, NT], f32, kind="ExternalInput").ap()
    bkr = nc.dram_tensor("bkr", [P, NT], f32, kind="ExternalInput").ap()
    bv = nc.dram_tensor("bv", [E], f32, kind="ExternalInput").ap()
    bo = nc.dram_tensor("bo", [E], f32, kind="ExternalInput").ap()
    xt_d = nc.dram_tensor("xt", [P, NT, L], bf16, kind="ExternalInput").ap()
    mask_d = nc.dram_tensor("mask01", [P, P], bf16, kind="ExternalInput").ap()
    out = nc.dram_tensor("out", [L, E], bf16, kind="ExternalOutput").ap()
    s_dram = nc.dram_tensor("s_scratch", [H, L], f32, kind="Internal").ap()

    with tile.TileContext(nc) as tc:
        _body(nc, tc, wq, wk, wv, wo, bqr, bkr, bv, bo, out, s_dram,
              xt_d, mask_d)
    return nc


def _body(nc, tc, wq, wk, wv, wo, bqr, bkr, bv, bo, out, s_dram, xt_d, mask_d):
    from contextlib import ExitStack

    ctx = ExitStack()
    with ctx:
        consts = ctx.enter_context(tc.tile_pool(name="consts", bufs=1))
        xt_pool = ctx.enter_context(tc.tile_pool(name="xt_pool", bufs=1))
        qt_pool = ctx.enter_context(tc.tile_pool(name="qt_pool", bufs=1))
        kt_pool = ctx.enter_context(tc.tile_pool(name="kt_pool", bufs=1))
        v_pool = ctx.enter_context(tc.tile_pool(name="v_pool", bufs=1))
        y_pool = ctx.enter_context(tc.tile_pool(name="y_pool", bufs=1))
        sst_pool = ctx.enter_context(tc.tile_pool(name="sst_pool", bufs=4))
        wblk_pool = ctx.enter_context(tc.tile_pool(name="wblk_pool", bufs=4))
        pt_pool = ctx.enter_context(tc.tile_pool(name="pt_pool", bufs=10))
        osb_pool = ctx.enter_context(tc.tile_pool(name="osb_pool", bufs=3))
        r_pool = ctx.enter_context(tc.tile_pool(name="r_pool", bufs=1))
        rh_pool = ctx.enter_context(tc.tile_pool(name="rh_pool", bufs=3))
        wo_pool = ctx.enter_context(tc.tile_pool(name="wo_pool", bufs=1))

        XT = xt_pool.tile([P, NT, L], bf16)  # [p, ct, l] = x^T[ct*128+p, l]
        QT = qt_pool.tile([P, NT, L], bf16)  # [p, et, l] = Q^T[et*128+p, l]
        KT = kt_pool.tile([P, NT, L], bf16)
        V = v_pool.tile([P, NT, H, D + 1], bf16)  # [p(l), lt, h, d | ones]
        Y = y_pool.tile([P, NT, L], bf16)  # [p, et, l] = y^T[et*128+p, l]
        Ybc = Y
        R = r_pool.tile([P, NT, L], f32)
        wo_r = wo_pool.tile([P, NT, E], bf16)

        # sync ring: XT chunks FIRST, then wo, then the small consts
        mask01 = consts.tile([P, P], bf16)
        ones_t = consts.tile([D + 1, P], bf16)
        bq_sb = consts.tile([P, NT], f32)
        bk_sb = consts.tile([P, NT], f32)
        bv_bc = consts.tile([P, E], f32)
        bo_bc = consts.tile([P, E], f32)

        nc.vector.memset(ones_t, 0.0)
        nc.vector.memset(ones_t[D : D + 1, :], 1.0)
        nc.vector.memset(V[:, :, :, D : D + 1], 1.0)
        nc.gpsimd.dma_start(
            out=bv_bc,
            in_=bass.AP(tensor=bv.tensor, offset=bv.offset, ap=[[0, P], [1, E]]),
        )

        # ---- Phase 1+2a: chunked XT/Wv loads + ct-outer V projection ----
        with tc.tile_pool(name="wv_pool", bufs=1) as wvp, \
             tc.tile_pool(name="vps", bufs=1, space="PSUM") as vps:
            wv_sb = wvp.tile([P, NT, E], bf16)
            for ct in range(NT):
                nc.sync.dma_start(out=XT[:, ct, :], in_=xt_d[:, ct, :])
                nc.scalar.dma_start(out=wv_sb[:, ct, :], in_=wv[:, ct, :])
            nc.sync.dma_start(out=wo_r, in_=wo)
            nc.sync.dma_start(out=mask01, in_=mask_d)
            nc.sync.dma_start(out=bq_sb, in_=bqr)
            nc.sync.dma_start(out=bk_sb, in_=bkr)
            for ec in range(2):
                psv = [
                    vps.tile([P, 512], f32, tag=f"v{lt}", name=f"psv{lt}")
                    for lt in range(NT)
                ]
                for ct in range(NT):
                    for lt in range(NT):
                        nc.tensor.matmul(
                            psv[lt],
                            XT[:, ct, lt * P : (lt + 1) * P],
                            wv_sb[:, ct, ec * 512 : (ec + 1) * 512],
                            start=(ct == 0),
                            stop=(ct == NT - 1),
                        )
                for lt in range(NT):
                    nc.vector.tensor_tensor(
                        out=V[:, lt, ec * 8 : (ec + 1) * 8, 0:D],
                        in0=psv[lt].rearrange("p (h d) -> p h d", h=8),
                        in1=bv_bc[:, ec * 512 : (ec + 1) * 512].rearrange(
                            "p (h d) -> p h d", h=8
                        ),
                        op=OP.add,
                    )

        # "big": 3x 2-bank slots shared by scores st / proj ps / out-proj
        # ps / ones-bcast; "yup": one AV accumulator bank per head (the
        # two q-halves are processed in sequence, reusing saved pt tiles)
        big = ctx.enter_context(tc.tile_pool(name="big", bufs=3, space="PSUM"))
        yup = ctx.enter_context(tc.tile_pool(name="yup", bufs=1, space="PSUM"))

        def proj_triggers(et):
            blks = []
            for w_dram in (wq, wk):
                wqk_blk = wblk_pool.tile(
                    [P, NT, P], bf16, tag="wqkblk", name="wqk_blk"
                )
                nc.scalar.dma_start(out=wqk_blk, in_=w_dram[et])
                blks.append(wqk_blk)
            return blks

        def proj_group(blks, g, et):
            # one of the 4 projection accumulation groups of e-tile et:
            # g = (q/k) * 2 + l-half — sized to hide one exp pair
            wi, lc = g // 2, g % 2
            b_sb, dst = (bq_sb, QT) if wi == 0 else (bk_sb, KT)
            ps = big.tile([P, L], f32, tag="st", name="ps_proj")
            for ct in range(NT):
                nc.tensor.matmul(
                    ps[:, 0:512],
                    blks[wi][:, ct, :],
                    XT[:, ct, lc * 512 : (lc + 1) * 512],
                    start=(ct == 0),
                    stop=(ct == NT - 1),
                )
            nc.vector.tensor_scalar(
                out=dst[:, et, lc * 512 : (lc + 1) * 512],
                in0=ps[:, 0:512],
                scalar1=b_sb[:, et : et + 1],
                scalar2=None,
                op0=OP.add,
            )

        def out_proj_lt(lt, woven=False):
            for oc in range(2):
                ps = big.tile([P, L], f32, tag="st", name="ps_out")
                for et in range(NT):
                    nc.tensor.matmul(
                        ps[:, 0:512],
                        Ybc[:, et, lt * P : (lt + 1) * P],
                        wo_r[:, et, oc * 512 : (oc + 1) * 512],
                        start=(et == 0),
                        stop=(et == NT - 1),
                    )
                osb = osb_pool.tile([P, 512], bf16)
                nc.vector.tensor_tensor(
                    out=osb,
                    in0=ps[:, 0:512],
                    in1=bo_bc[:, oc * 512 : (oc + 1) * 512],
                    op=OP.add,
                )
                eng = nc.sync if (lt + oc) % 2 == 0 else nc.scalar
                eng.dma_start(
                    out=out[lt * P : (lt + 1) * P, oc * 512 : (oc + 1) * 512],
                    in_=osb,
                )

        # ---- Phase 2b+3: proj(0) prologue, then per-et attention pairs ----
        blocks = {}
        blocks[0] = proj_triggers(0)
        for g in range(4):
            proj_group(blocks[0], g, 0)
        blocks[1] = proj_triggers(1)

        def trig(e):
            blocks[e] = proj_triggers(e)

        for et in range(NT):
            last_pair = et == NT - 1
            if last_pair:
                fillers = {
                    4 + i: (lambda lt=i: out_proj_lt(lt, woven=True))
                    for i in range(4)
                }
            else:
                # next e-tile's projection split across the odd rounds so
                # every AV's exp latency hides behind a matmul group;
                # weight DMA triggers fire two pairs ahead
                fillers = {
                    1: (lambda e=et + 1: proj_group(blocks[e], 0, e)),
                    3: (lambda e=et + 1: proj_group(blocks[e], 1, e)),
                    5: (lambda e=et + 1: proj_group(blocks[e], 2, e)),
                    7: (lambda e=et + 1: proj_group(blocks[e], 3, e)),
                }
                if et + 2 < NT:
                    fillers[0] = lambda e=et + 2: trig(e)
            _attention_pair(
                nc, et, QT, KT, V, Ybc, s_dram, big, yup, pt_pool,
                sst_pool, mask01, last_pair, ones_t, rh_pool, fillers,
            )
            if not last_pair:
                for half in range(2):
                    hh = 2 * et + half
                    bsrc = bass.AP(
                        tensor=s_dram.tensor,
                        offset=s_dram[hh : hh + 1, :].offset,
                        ap=[[0, 64], [1, L]],
                    )
                    nc.gpsimd.dma_start(
                        out=R[half * 64 : (half + 1) * 64, et, :], in_=bsrc
                    )
                nc.vector.reciprocal_approx_fast(out=R[:, et, :], in_=R[:, et, :])
                for half in range(2):
                    rows = slice(half * 64, (half + 1) * 64)
                    nc.vector.tensor_tensor(
                        out=Ybc[rows, et, :],
                        in0=Y[rows, et, :],
                        in1=R[rows, et, :],
                        op=OP.mult,
                    )
                if et == 0:
                    # deferred: not needed before the out-proj, keeps the
                    # SWDGE queue clear of the head's XT/Wv window
                    nc.gpsimd.dma_start(
                        out=bo_bc,
                        in_=bass.AP(
                            tensor=bo.tensor, offset=bo.offset,
                            ap=[[0, P], [1, E]],
                        ),
                    )

        # ---- Phase 5 tail: remaining out-proj l-tiles ----
        for lt in range(4, NT):
            out_proj_lt(lt)


def _attention_pair(nc, et, QT, KT, V, Ybc, s_dram, big, yup, pt_pool,
                    sst_pool, mask01, last_pair, ones_t, rh_pool, fillers):
    """Both heads (2et, 2et+1) of e-tile et.

    Rounds of [S(kt) pair, filler, AV jobs]: the AV of a k-tile runs a
    round after its exp, so ScalarE latency hides behind the next
    scores/filler matmuls. The two q-halves are accumulated in
    SEQUENCE: qc0 closes at kt=3, then qc1 accumulates kt=0..7 reusing
    the saved pt tiles (no second exp) — so only one PSUM accumulator
    bank per head is live at a time.
    """
    heads = (2 * et, 2 * et + 1)
    yu = {}

    def alloc_yu(qc):
        for hi in range(2):
            yu[(hi, qc)] = yup.tile(
                [D + 1, 512], f32, tag=f"yu{hi}", name=f"yu{hi}{qc}"
            )

    pts_store = {}

    def S_round(kt):
        qlen = L - kt * P
        pts = []
        for hi in range(2):
            pb = hi * 64
            st = big.tile([P, L], f32, tag="st", name="st")
            for s0 in range(0, qlen, 512):
                n = min(512, qlen - s0)
                nc.tensor.matmul(
                    st[:, s0 : s0 + n],
                    KT[pb : pb + D, et, kt * P : (kt + 1) * P],
                    QT[pb : pb + D, et, kt * P + s0 : kt * P + s0 + n],
                    start=True,
                    stop=True,
                )
            pt = pt_pool.tile([P, L], bf16, tag="pt", name="pt")
            nc.scalar.activation(
                out=pt[:, 0:qlen], in_=st[:, 0:qlen], func=AF.Exp,
                scale=float(SCALE),
            )
            nc.vector.tensor_tensor(
                out=pt[:, 0:P], in0=pt[:, 0:P], in1=mask01, op=OP.mult
            )
            pts.append(pt)
        pts_store[kt] = pts

    def A(qc, kt):
        lo = max(qc * 512, kt * P)
        hi_col = (qc + 1) * 512
        last_kt = 3 if qc == 0 else NT - 1
        for hi, h in enumerate(heads):
            nc.tensor.matmul(
                yu[(hi, qc)][:, lo - qc * 512 : hi_col - qc * 512],
                V[:, kt, h, :],
                pts_store[kt][hi][:, lo - kt * P : hi_col - kt * P],
                start=(kt == 0),
                stop=(kt == last_kt),
            )
            if kt == last_kt:
                _head_tail(nc, h, hi, qc, yu, Ybc, s_dram, sst_pool,
                           last_pair, ones_t, rh_pool, big)

    def F(kt):
        if kt in fillers:
            fillers[kt]()

    alloc_yu(0)
    S_round(0); F(0)
    S_round(1); F(1); A(0, 0)
    S_round(2); F(2); A(0, 1)
    S_round(3); F(3); A(0, 2)
    S_round(4); A(0, 3); alloc_yu(1); A(1, 0); F(4)
    S_round(5); A(1, 1); A(1, 2); F(5)
    S_round(6); A(1, 3); A(1, 4); F(6)
    S_round(7); A(1, 5); A(1, 6); F(7)
    A(1, 7)


def _head_tail(nc, h, hi, qc, yu, Ybc, s_dram, sst_pool, last_pair, ones_t,
               rh_pool, big):
    et = h // 2
    pb = hi * 64
    cols = slice(qc * 512, (qc + 1) * 512)
    yuq = yu[(hi, qc)]
    if last_pair:
        sstb = sst_pool.tile([D + 1, 512], bf16, tag="sstb", name="sstb")
        nc.vector.tensor_copy(out=sstb[D : D + 1, :], in_=yuq[D : D + 1, :])
        ps_bc = big.tile([P, L], f32, tag="st", name="ps_bc")
        nc.tensor.matmul(
            ps_bc[:, 0:512], ones_t[D : D + 1, :], sstb[D : D + 1, :],
            start=True, stop=True,
        )
        rh = rh_pool.tile([P, 512], f32, tag="rh", name="rh")
        nc.vector.reciprocal_approx_fast(out=rh, in_=ps_bc[:, 0:512])
        nc.vector.tensor_tensor(
            out=Ybc[pb : pb + D, et, cols],
            in0=yuq[0:D, :],
            in1=rh[0:D, :],
            op=OP.mult,
        )
    else:
        sst = sst_pool.tile([D + 1, 512], f32, tag="sst", name="sst")
        nc.vector.tensor_copy(out=sst[D : D + 1, :], in_=yuq[D : D + 1, :])
        nc.sync.dma_start(out=s_dram[h : h + 1, cols], in_=sst[D : D + 1, :])
        nc.vector.tensor_copy(out=Ybc[pb : pb + D, et, cols], in_=yuq[0:D, :])


_COMPILED = None


def _get_compiled():
    global _COMPILED
    if _COMPILED is None:
        nc = _build()
        nc.compile()
        _COMPILED = nc
    return _COMPILED


def kernel(x, Wq, bq, Wk, bk, Wv, bv, Wo, bo, _trace=False):
    import ml_dtypes

    bfl = ml_dtypes.bfloat16
    nc = _get_compiled()
    x = np.ascontiguousarray(np.asarray(x, dtype=np.float32).astype(bfl))
    B = x.shape[0]
    assert B == 8 and x.shape[1] == L and x.shape[2] == E
    def _qk_layout(w):
        # [et, p, ct, e']: per-et contiguous [128, 8, 128] stationary blocks
        w = np.asarray(w, np.float32).astype(bfl)
        return np.ascontiguousarray(
            w.reshape(NT, P, NT, P).transpose(2, 1, 0, 3)
        )

    def _pct_layout(w):
        # [p, ct, e]: moving-operand blocks with contraction rows on partitions
        w = np.asarray(w, np.float32).astype(bfl)
        return np.ascontiguousarray(w.reshape(NT, P, E).transpose(1, 0, 2))

    def _b_rearranged(b):
        # [p, et]: per-partition bias rows, contiguous for a fast DMA
        return np.ascontiguousarray(
            np.asarray(b, np.float32).reshape(NT, P).T
        )

    common = {
        "wq": _qk_layout(Wq),
        "wk": _qk_layout(Wk),
        "wv": _pct_layout(Wv),
        "wo": _pct_layout(Wo),
        "bqr": _b_rearranged(bq),
        "bkr": _b_rearranged(bk),
        "bv": np.ascontiguousarray(np.asarray(bv, np.float32)),
        "bo": np.ascontiguousarray(np.asarray(bo, np.float32)),
    }
    common["mask01"] = np.tril(np.ones((P, P), np.float32)).T.astype(bfl)
    # xt[b]: [p, ct, l] with xt[b][p, ct, l] = x[b, l, ct*128+p]
    xt = np.ascontiguousarray(
        x.transpose(0, 2, 1).reshape(B, NT, P, L).transpose(0, 2, 1, 3)
    )
    in_maps = [dict(common, xt=xt[i]) for i in range(B)]
    res = run_bass_kernel_spmd(nc, in_maps, core_ids=list(range(8)), trace=_trace)
    outp = np.stack(
        [np.asarray(res.results[i]["out"]).astype(np.float32) for i in range(B)]
    )
    if _trace:
        kernel.last_exec_time_ns = res.exec_time_ns
    return outp
